# revision 28
# baseline (speedup 1.0000x reference)
"""Trainium2 Bass kernel for DocumentGraphEncoder (3-layer GATv2 + LN + gated pooling).

Self-contained: takes FULL inputs, shards across 8 NeuronCores internally,
returns FULL [64, 256] float32 output.

Sharding: nodes partitioned contiguously across 8 cores (3750/core), assigned
to 32 groups of 120 node slots (rows g*128+slot, slots 120-127 reserved).
Each core owns the edges whose dst is in its range, sorted by (dst_group,
dst, src-gather-index) and padded so every dst-group is a multiple of 512
edges. Per layer: dense transforms run node-major (xl pass, AllGather fp8,
xr pass overlapped with the collective), per-edge source rows arrive via
SWDGE dma_gather from the AllGathered table, z = xr[dst]+e+xl[src] is built
with PE matmuls (edge-attr transform folded into the dst-scatter matmul via
stationary partitions 120-127), leaky-relu runs natively on the scalar
engine, segment softmax/scatter are PE matmuls with PSUM accumulation. The
edge phase is software-pipelined: gather/E1(z+lrelu) of group g overlaps
logits/scatter/normalize of group g-1. Final graph pooling is a per-group
matmul + AllReduce of [64, 257] partials.
"""
import numpy as np
import os as _os
from contextlib import ExitStack

import concourse.bass as bass
import concourse.bacc as bacc
import concourse.tile as tile
import concourse.mybir as mybir
from concourse._compat import get_trn_type, cdiv
from concourse.bass_utils import run_bass_kernel_spmd

FP16 = mybir.dt.float16
FP8 = mybir.dt.float8e4
F32 = mybir.dt.float32
I16 = mybir.dt.int16
AFT = mybir.ActivationFunctionType
ALU = mybir.AluOpType

N, E, IN, HID, G = 30000, 480000, 399, 256, 64
NEG = 0.2
NCORE = 8
NLOC = N // NCORE          # 3750
GPN = 120                  # node slots per group; 120-127 carry we rows
GP = 128
NGRP = cdiv(NLOC, GPN)     # 32
NLOCP = NGRP * GP          # 4096
NP = NCORE * NLOCP         # 32768 (int16 gather index max 32767 - fits)
KB1 = 4                    # 512 = padded IN contraction blocks
HEADS = (8, 8, 1)
LN_EPS = 1e-5
DEN_EPS = 1e-30

N_LAYERS = int(_os.environ.get("K_LAYERS", "3"))
NGRP_USE = int(_os.environ.get("K_NGRP", str(NGRP)))

_prog_cache = {}


def _wrap_idx(idx, egrp):
    """[..., EGRP] int16 -> wrapped [., 128, EGRP//16] layout for dma_gather."""
    lead = idx.shape[:-1]
    w = np.zeros(lead + (128, egrp // 16), np.int16)
    r = idx.reshape(lead + (egrp // 16, 16))
    for rep in range(8):
        w[..., rep * 16:(rep + 1) * 16, :] = np.swapaxes(r, -1, -2)
    return w


def _host_prep(inputs):
    x = np.asarray(inputs["x"], np.float32)
    edge_index = np.asarray(inputs["edge_index"], np.int64)
    edge_attr = np.asarray(inputs["edge_attr"], np.float32)
    batch = np.asarray(inputs["batch"], np.int64)
    src, dst = edge_index[0], edge_index[1]

    import heapq
    core_of = dst // NLOC
    per_core = []
    perms = []
    maxgrp = 0
    for c in range(NCORE):
        m = np.nonzero(core_of == c)[0]
        ld0 = dst[m] - c * NLOC
        deg = np.bincount(ld0, minlength=NLOC)
        # LPT: assign nodes (desc degree) to least-loaded group with space
        order_n = np.argsort(-deg, kind="stable")
        heap = [(0, 0, gi) for gi in range(NGRP)]
        heapq.heapify(heap)
        perm = np.empty(NLOC, np.int64)
        for node in order_n:
            load, fill, gi = heapq.heappop(heap)
            perm[node] = gi * GP + fill
            if fill + 1 < GPN:
                heapq.heappush(heap, (load + int(deg[node]), fill + 1, gi))
        perms.append(perm)
        ld = perm[ld0]
        g = ld // GP
        order = np.lexsort((src[m], ld))
        m, s, ld, g = m[order], src[m][order], ld[order], g[order]
        cnt = np.bincount(g, minlength=NGRP)
        maxgrp = max(maxgrp, int(cnt.max()))
        per_core.append((m, s, ld, g, cnt))
    egrp = cdiv(maxgrp, 128) * 128
    nchk = egrp // 128

    all_perm = np.stack(perms)
    gmax = np.zeros(NGRP, np.int64)
    for c in range(NCORE):
        gmax = np.maximum(gmax, per_core[c][4])
    nchk_gs = [int(cdiv(int(v), 128)) for v in gmax]
    host = {"egrp": egrp, "nchk": nchk, "nchk_gs": nchk_gs, "cores": []}
    for c in range(NCORE):
        m, s, ld, g, cnt = per_core[c]
        src_pad = np.zeros((NGRP, egrp), np.int64)
        oh_em = np.zeros((NGRP, 128, nchk, 128), mybir.dt.np(FP8))
        oh_nm = np.zeros((NGRP, 128, nchk, 128), mybir.dt.np(FP8))
        off = np.concatenate([[0], np.cumsum(cnt)])
        for gg in range(NGRP):
            n_e = int(cnt[gg])
            sl = slice(off[gg], off[gg] + n_e)
            sg, ldg, mg = s[sl], ld[sl], m[sl]
            sc = sg // NLOC
            gidx = NLOCP * sc + all_perm[sc, sg % NLOC]
            # sort edges by gather index for HBM locality in dma_gather
            ord2 = np.argsort(gidx, kind="stable")
            sg, ldg, mg, gidx = sg[ord2], ldg[ord2], mg[ord2], gidx[ord2]
            src_pad[gg, :n_e] = gidx
            rel = (ldg - gg * GP).astype(np.int64)
            ee = np.arange(n_e)
            oh_em[gg, ee % 128, ee // 128, rel] = 1.0
            oh_nm[gg, rel, ee // 128, ee % 128] = 1.0
            # edge-attr rows in the we-partitions (120..124)
            ea = edge_attr[mg].T.astype(np.float32)  # [4, n_e]
            for j in range(4):
                oh_nm[gg, 120 + j, ee // 128, ee % 128] = ea[j].astype(
                    mybir.dt.np(FP8))
            oh_nm[gg, 124, ee // 128, ee % 128] = 1.0
        pc = all_perm[c]
        xs = np.zeros((NLOCP, 512), np.float32)
        xs[pc, :IN] = x[c * NLOC:(c + 1) * NLOC]
        bo = np.zeros((NGRP, GP, G), np.float16)
        bo[pc // GP, pc % GP, batch[c * NLOC:(c + 1) * NLOC]] = 1.0
        host["cores"].append({
            "xT": np.ascontiguousarray(xs.T).astype(np.float16),
            "src_idx": _wrap_idx(src_pad.astype(np.int16), egrp),
            "oh_em": oh_em,
            "oh_nm": oh_nm,
            "bonehot": bo,
        })

    # weights
    def f16(a):
        return np.asarray(a, np.float32).astype(np.float16)

    wmeta = {}
    dims = [(IN, 8, 32), (HID, 8, 32), (HID, 1, 256)]
    for li, (fin, h, cdim) in enumerate(dims, 1):
        kb = KB1 if li == 1 else 2
        wl = np.zeros((kb * 128, 256), np.float32)
        wr = np.zeros((kb * 128, 256), np.float32)
        wl[:fin] = np.asarray(inputs[f"wl{li}"], np.float32)
        wr[:fin] = np.asarray(inputs[f"wr{li}"], np.float32)
        we = np.asarray(inputs[f"we{li}"], np.float32)
        bl = np.asarray(inputs[f"bl{li}"], np.float32)
        br = np.asarray(inputs[f"br{li}"], np.float32)
        # stationary rows 120..127 of each group's xr block: we rows + bias
        webc = np.zeros((8, NGRP, 256), np.float16)
        webc[:4] = f16(we)[:, None, :]
        webc[4] = f16(bl + br)[None, :]
        att = np.asarray(inputs[f"att{li}"], np.float32)  # [h, cdim]
        blk = np.zeros((256, 8), np.float32)
        for hh in range(h):
            blk[hh * cdim:(hh + 1) * cdim, hh] = att[hh]
        attz = np.stack([f16(blk[:128]), f16(blk[128:])])
        nbias = np.tile((np.asarray(inputs[f"b{li}"], np.float32)
                         + bl).astype(np.float16), (128, 1))
        # moving operand for node-major dense: per (t, k) block [128, 256]
        wcat = np.zeros((128, 2 * kb * 256), np.float16)
        for t, w in enumerate((wl, wr)):
            for k in range(kb):
                base = (t * kb + k) * 256
                wcat[:, base:base + 256] = f16(w[k * 128:(k + 1) * 128])
        wmeta[li] = dict(kb=kb, h=h, wcat=wcat, webc=webc, attz=attz,
                         nbias=nbias)

    consts = {
        "id128": np.eye(128, dtype=np.float16),
        "id8": np.eye(8, dtype=np.float16),
        "id64": np.eye(64, dtype=np.float32),
        "epsden": np.full((128, 1), DEN_EPS, np.float32),
        "lnw": np.tile(np.asarray(inputs["ln_w"], np.float32), (128, 1)),
        "lnb": np.tile(np.asarray(inputs["ln_b"], np.float32), (128, 1)),
        "gatew": np.tile(np.asarray(inputs["gate_w"], np.float32)[:, 0]
                         .astype(np.float16), (128, 1)),
        "gateb": np.full((128, 1), float(np.asarray(inputs["gate_b"])[0]), np.float32),
        "trw": np.stack([np.asarray(inputs["tr_w"], np.float32)[:128],
                         np.asarray(inputs["tr_w"], np.float32)[128:]]),
        "trb": np.tile(np.asarray(inputs["tr_b"], np.float32), (64, 1)),
    }
    host["wmeta"] = wmeta
    host["consts"] = consts
    return host


def _build_program(egrp, nchk, wmeta_shapes, nchk_gs):
    nc = bacc.Bacc(get_trn_type() or "TRN2", target_bir_lowering=False,
                   debug=False, num_swdge_queues=4)

    # ---- external inputs ----
    xT_in = nc.dram_tensor("xT", [512, NLOCP], FP16, kind="ExternalInput")
    sidx_in = nc.dram_tensor("src_idx", [NGRP, 128, egrp // 16], I16, kind="ExternalInput")
    ohem_in = nc.dram_tensor("oh_em", [NGRP, 128, nchk, 128], FP8, kind="ExternalInput")
    ohnm_in = nc.dram_tensor("oh_nm", [NGRP, 128, nchk, 128], FP8, kind="ExternalInput")
    bo_in = nc.dram_tensor("bonehot", [NGRP, 128, G], FP16, kind="ExternalInput")
    w_in = {}
    for li in (1, 2, 3):
        kb = wmeta_shapes[li]
        w_in[li] = dict(
            wcat=nc.dram_tensor(f"wcat{li}", [128, 2 * kb * 256], FP16, kind="ExternalInput"),
            webc=nc.dram_tensor(f"webc{li}", [8, NGRP, 256], FP16, kind="ExternalInput"),
            attz=nc.dram_tensor(f"attz{li}", [2, 128, 8], FP16, kind="ExternalInput"),
            nbias=nc.dram_tensor(f"nbias{li}", [128, 256], FP16, kind="ExternalInput"),
        )
    _NOPRELOAD = ("trw",)
    cin = {k: nc.dram_tensor(k, list(v.shape),
                             FP16 if v.dtype == np.float16 else F32,
                             kind="ExternalInput")
           for k, v in {
               "id128": np.zeros((128, 128), np.float16),
               "id8": np.zeros((8, 8), np.float16),
               "id64": np.zeros((64, 64), np.float32),
               "epsden": np.zeros((128, 1), np.float32),
               "lnw": np.zeros((128, 256), np.float32),
               "lnb": np.zeros((128, 256), np.float32),
               "gatew": np.zeros((128, 256), np.float16),
               "gateb": np.zeros((128, 1), np.float32),
               "trw": np.zeros((2, 128, 256), np.float32),
               "trb": np.zeros((64, 256), np.float32),
           }.items()}
    out_t = nc.dram_tensor("out", [G, HID], F32, kind="ExternalOutput")
    ABL = _os.environ.get("K_ABL", "")

    RG = [list(range(NCORE))]

    with tile.TileContext(nc) as tc, ExitStack() as octx:
        dram = octx.enter_context(tc.tile_pool(name="dram", bufs=1, space="DRAM"))
        xl_loc = dram.tile([NLOCP, 256], FP8)
        xl_fulls = [dram.tile([NP, 256], FP8, addr_space="Shared",
                              name=f"xl_full{i}") for i in range(3)]
        pre_in_d = dram.tile([G, 257], F32)
        pre_out_d = dram.tile([G, 257], F32, addr_space="Shared", name="pre_out")

        cpool = octx.enter_context(tc.tile_pool(name="const", bufs=1))
        csb = {}
        for k, t in cin.items():
            if k in _NOPRELOAD:
                continue
            csb[k] = cpool.tile(list(t.shape), t.dtype, name=f"c_{k}")
            nc.sync.dma_start(csb[k][:], t[:])
        bo_sb = cpool.tile([128, NGRP, G], FP16)
        nc.sync.dma_start(bo_sb[:], bo_in[:].rearrange("g p b -> p g b"))
        idx_all = cpool.tile([128, NGRP, egrp // 16], I16)
        nc.sync.dma_start(idx_all[:], sidx_in[:].rearrange("g p e -> p g e"))

        persist = octx.enter_context(tc.tile_pool(name="persist", bufs=1))
        xr_nm = persist.tile([128, NGRP, 256], FP16)
        h_ln = persist.tile([128, NGRP, 256], FP16)
        pre_acc = persist.tile([G, 257], F32)
        # h of the previous layer, feature-major (filled incrementally by the
        # edge-loop normalize stage via PE transposes)
        hT_p = persist.tile([128, 2, NLOCP], FP16)
        # xl weight blocks of layers 2/3 for the in-edge-loop xl pass
        wxl = {}
        for li in (2, 3):
            wxl[li] = persist.tile([128, 2, 256], FP16, name=f"wxl{li}")
            nc.sync.dma_start(wxl[li][:], w_in[li]["wcat"][:, 0:512]
                              .rearrange("p (k m) -> p k m", k=2))

        nc.vector.memset(pre_acc[:], 0.0)
        for li in range(1, N_LAYERS + 1):
            kb = wmeta_shapes[li]
            hh = HEADS[li - 1]
            wt = w_in[li]
            ln = f"{li}"

            # ============ dense phase (node-major direct) ============
            with ExitStack() as lctx:
                dp = lctx.enter_context(tc.tile_pool(name=f"d{ln}", bufs=1))
                dps = lctx.enter_context(tc.tile_pool(name=f"dps{ln}", bufs=4, space="PSUM"))
                stg = lctx.enter_context(tc.tile_pool(name=f"stg{ln}", bufs=3))

                w_sb = dp.tile([128, 2 * kb * 256], FP16)
                nc.sync.dma_start(w_sb[:], wt["wcat"][:])

                def wmov(t, k):
                    base = (t * kb + k) * 256
                    return w_sb[:, base:base + 256]

                if li == 1:
                    # layer 1 reads x directly (kb=4 contraction blocks); the
                    # xl pass runs first so the AllGather launches early.
                    hT = dp.tile([128, kb, NLOCP], FP16)
                    for k in range(kb):
                        nc.sync.dma_start(hT[:, k, :], xT_in[k * 128:(k + 1) * 128, :])
                    for gg in range(NGRP):
                        nsl = slice(gg * 128, (gg + 1) * 128)
                        psl = dps.tile([128, 256], F32, name="ps_xl")
                        for k in range(kb):
                            nc.tensor.matmul(psl[:], hT[:, k, nsl], wmov(0, k),
                                             start=(k == 0), stop=(k == kb - 1))
                        xls = stg.tile([128, 256], FP8, name="st_xl")
                        nc.scalar.activation(xls[:], psl[:], AFT.Copy)
                        nc.sync.dma_start(xl_loc[nsl, :], xls[:])
                else:
                    # xl for this layer was already produced inside the
                    # previous layer's edge loop (hT_p + xl_loc stores).
                    hT = hT_p

                if ABL != "noag":
                    nc.gpsimd.collective_compute(
                        "AllGather", ALU.bypass, replica_groups=RG,
                        ins=[xl_loc[:].opt()], outs=[xl_fulls[li - 1][:].opt()])

                nc.sync.dma_start(xr_nm[120:128, :, :], wt["webc"][:])
                for gg in range(NGRP):
                    nsl = slice(gg * 128, (gg + 1) * 128)
                    psr = dps.tile([128, 256], F32, name="ps_xr")
                    for k in range(kb):
                        nc.tensor.matmul(psr[:], hT[:, k, nsl], wmov(1, k),
                                         start=(k == 0), stop=(k == kb - 1))
                    nc.scalar.activation(xr_nm[0:120, gg, :], psr[0:120, :], AFT.Copy)

            # ======== edge phase (software-pipelined groups) ========
            with ExitStack() as lctx:
                ep = lctx.enter_context(tc.tile_pool(name=f"e{ln}", bufs=3))
                lp = lctx.enter_context(tc.tile_pool(name=f"l{ln}", bufs=8))
                gbuf = lctx.enter_context(tc.tile_pool(name=f"g{ln}", bufs=8))
                upool = lctx.enter_context(tc.tile_pool(name=f"u{ln}", bufs=2))
                mpool = lctx.enter_context(tc.tile_pool(name=f"m{ln}", bufs=2))
                epz = lctx.enter_context(tc.tile_pool(name=f"ez{ln}", bufs=2, space="PSUM"))
                epl = lctx.enter_context(tc.tile_pool(name=f"el{ln}", bufs=1, space="PSUM"))
                epe = lctx.enter_context(tc.tile_pool(name=f"ee{ln}", bufs=1, space="PSUM"))
                epp = lctx.enter_context(tc.tile_pool(name=f"ep{ln}", bufs=1, space="PSUM"))
                epa = lctx.enter_context(tc.tile_pool(name=f"ea{ln}", bufs=2, space="PSUM"))
                dxl = lctx.enter_context(tc.tile_pool(name=f"x{ln}", bufs=2, space="PSUM"))
                wp = lctx.enter_context(tc.tile_pool(name=f"w{ln}", bufs=1))

                attz_sb = wp.tile([128, 2, 8], FP16)
                nc.sync.dma_start(attz_sb[:], wt["attz"][:].rearrange("f p h -> p f h"))
                nbias_sb = wp.tile([128, 256], FP16)
                nc.sync.dma_start(nbias_sb[:], wt["nbias"][:])

                state = {}
                stateB = {}
                stateC = {}

                def load_stage(gg):
                    nchk_g = nchk_gs[gg]
                    egrp_g = nchk_g * 128
                    xg = gbuf.tile([128, nchk, 256], FP8, name="xg")
                    if ABL not in ("nogather", "nothing"):
                        nc.gpsimd.dma_gather(xg[:, :nchk_g, :], xl_fulls[li - 1][:],
                                             idx_all[:, gg, :egrp_g // 16], egrp_g,
                                             egrp_g, 256, single_packet=False,
                                             queue_num=gg % 4)
                    else:
                        nc.vector.memset(xg[:], 0.25)
                    ohe_sb = lp.tile([128, nchk, 128], FP8, name="ohe")
                    ohn_sb = lp.tile([128, nchk, 128], FP8, name="ohn")
                    if ABL not in ("noload", "nothing"):
                        nc.sync.dma_start(ohe_sb[:, :nchk_g, :],
                                          ohem_in[gg, :, :nchk_g, :])
                        nc.sync.dma_start(ohn_sb[:, :nchk_g, :],
                                          ohnm_in[gg, :, :nchk_g, :])
                    else:
                        nc.vector.memset(ohe_sb[:], 0.01)
                        nc.vector.memset(ohn_sb[:], 0.01)
                    state[gg] = [xg, ohe_sb, ohn_sb, None, nchk_g]

                def tiles_of(nchk_g):
                    return [(t, min(4, nchk_g - 4 * t)) for t in range(cdiv(nchk_g, 4))]

                def e1_stage(gg):
                    xg, ohe_sb, ohn_sb, _, nchk_g = state[gg]
                    u_all = upool.tile([128, cdiv(nchk, 4), 2, 512], FP16, name="uall")
                    state[gg][3] = u_all
                    for t, csz in tiles_of(nchk_g):
                        w = csz * 128
                        for fb in range(2):
                            pz = epz.tile([128, 512], F32, name="pz")
                            nc.tensor.matmul(pz[:, :w], xr_nm[:, gg, fb * 128:(fb + 1) * 128],
                                             ohn_sb[:, 4 * t:4 * t + csz, :],
                                             start=True, stop=False)
                            for c4 in range(csz):
                                nc.tensor.matmul(pz[:, c4 * 128:(c4 + 1) * 128],
                                                 xg[:, t * 4 + c4, fb * 128:(fb + 1) * 128],
                                                 csb["id128"][:], start=False,
                                                 stop=(c4 == csz - 1))
                            nc.scalar.activation(u_all[:, t, fb, :w],
                                                 pz[:, :w], AFT.Prelu, alpha=NEG)

                def e23_stage(gg):
                    xg, ohe_sb, ohn_sb, u_all, nchk_g = state.pop(gg)
                    msgall = mpool.tile([128, nchk, 264], FP16, name="msgall")
                    # E2: logits + exp + per-chunk exp transpose
                    for t, csz in tiles_of(nchk_g):
                        w = csz * 128
                        pl = epl.tile([hh, 512], F32, name="pl")
                        for fb in range(2):
                            nc.tensor.matmul(pl[:, :w], attz_sb[:, fb, :hh],
                                             u_all[:, t, fb, :w],
                                             start=(fb == 0), stop=(fb == 1))
                        expT = ep.tile([hh, 512], FP16, name="expT")
                        nc.scalar.activation(expT[:, :w], pl[:, :w], AFT.Exp)
                        pse = epe.tile([128, 4, 8], F32, name="pse")
                        for c4 in range(csz):
                            nc.tensor.matmul(pse[:, c4, :hh],
                                             expT[:, c4 * 128:(c4 + 1) * 128],
                                             csb["id8"][:hh, :hh], start=True, stop=True)
                        nc.scalar.activation(
                            msgall[:, t * 4:t * 4 + csz, 256:256 + hh],
                            pse[:, :csz, :hh], AFT.Copy)
                    # E3: messages + scatter
                    for t, csz in tiles_of(nchk_g):
                        if hh == 8:
                            ebc = msgall[:, t * 4:t * 4 + csz, 256:264][:, :, :, None] \
                                .broadcast_to([128, csz, 8, 32])
                        else:
                            ebc = msgall[:, t * 4:t * 4 + csz, 256:257][:, :, :, None] \
                                .broadcast_to([128, csz, 1, 256])
                        nc.vector.tensor_mul(
                            msgall[:, t * 4:t * 4 + csz, :256]
                                .rearrange("p c (h w) -> p c h w", h=hh),
                            xg[:, t * 4:t * 4 + csz, :256]
                                .rearrange("p a (h w) -> p a h w", h=hh),
                            ebc)
                    acc = epa.tile([128, 264], F32, name="acc")
                    for c4 in range(nchk_g):
                        nc.tensor.matmul(acc[:], ohe_sb[:, c4, :],
                                         msgall[:, c4, :],
                                         start=(c4 == 0),
                                         stop=(c4 == nchk_g - 1))
                    stateB[gg] = acc

                def norm_stage(gg):
                    acc = stateB.pop(gg)
                    nsl = slice(gg * 128, (gg + 1) * 128)
                    den = ep.tile([128, 8], F32, name="den")
                    nc.scalar.activation(den[:, :hh], acc[:, 256:256 + hh], AFT.Identity,
                                         bias=csb["epsden"][:])
                    rec = ep.tile([128, 8], F32, name="rec")
                    nc.vector.reciprocal(rec[:, :hh], den[:, :hh])
                    if li < 3:
                        h0 = ep.tile([128, 256], FP16, name="h0")
                        rbc = (rec[:, :hh][:, :, None].broadcast_to([128, hh, 256 // hh]))
                        nc.vector.tensor_mul(
                            h0[:].rearrange("p (h w) -> p h w", h=hh),
                            acc[:, :256].rearrange("p (h w) -> p h w", h=hh), rbc)
                        hb = ep.tile([128, 256], FP16, name="hb")
                        nc.vector.tensor_add(hb[:], h0[:], nbias_sb[:])
                        r_ = ep.tile([128, 256], FP16, name="relu")
                        nc.vector.tensor_scalar_max(r_[:], hb[:], 0.0)
                        nm = ep.tile([128, 256], FP16, name="nmin")
                        nc.vector.tensor_scalar_min(nm[:], hb[:], 0.0)
                        en = ep.tile([128, 256], FP16, name="expn")
                        nc.scalar.activation(en[:], nm[:], AFT.Exp)
                        h2 = ep.tile([128, 256], FP16, name="h2")
                        nc.vector.tensor_add(h2[:], r_[:], en[:])
                        hf = ep.tile([128, 256], FP16, name="hf")
                        nc.vector.tensor_scalar_add(hf[:], h2[:], -1.0)
                        stateC[gg] = hf
                    else:
                        h0 = ep.tile([128, 256], F32, name="h0f")
                        rbc = rec[:, :1][:, :, None].broadcast_to([128, 1, 256])
                        nc.vector.tensor_mul(
                            h0[:].rearrange("p (h w) -> p h w", h=1),
                            acc[:, :256].rearrange("p (h w) -> p h w", h=1), rbc)
                        hb = ep.tile([128, 256], F32, name="hbf")
                        nc.vector.tensor_add(hb[:], h0[:], nbias_sb[:])
                        mu = ep.tile([128, 1], F32, name="mu")
                        nc.vector.reduce_sum(mu[:], hb[:], axis=mybir.AxisListType.X)
                        nmu = ep.tile([128, 1], F32, name="nmu")
                        nc.vector.tensor_scalar_mul(nmu[:], mu[:], -1.0 / 256.0)
                        cent = ep.tile([128, 256], F32, name="cent")
                        nc.scalar.activation(cent[:], hb[:], AFT.Identity, bias=nmu[:])
                        sq = ep.tile([128, 256], F32, name="sq")
                        ssq = ep.tile([128, 1], F32, name="ssq")
                        nc.scalar.activation(sq[:], cent[:], AFT.Square, accum_out=ssq[:])
                        var = ep.tile([128, 1], F32, name="var")
                        nc.vector.tensor_scalar(var[:], ssq[:], 1.0 / 256.0, LN_EPS,
                                                op0=ALU.mult, op1=ALU.add)
                        sd = ep.tile([128, 1], F32, name="sd")
                        nc.scalar.activation(sd[:], var[:], AFT.Sqrt)
                        rstd = ep.tile([128, 1], F32, name="rstd")
                        nc.vector.reciprocal(rstd[:], sd[:])
                        lnt = ep.tile([128, 256], F32, name="lnt")
                        nc.vector.tensor_scalar_mul(lnt[:], cent[:], rstd[:])
                        lnt2 = ep.tile([128, 256], F32, name="lnt2")
                        nc.vector.tensor_mul(lnt2[:], lnt[:], csb["lnw"][:])
                        nc.vector.tensor_add(h_ln[:, gg, :], lnt2[:], csb["lnb"][:])
                        gm = ep.tile([128, 256], FP16, name="gm")
                        nc.vector.tensor_mul(gm[:], h_ln[:, gg, :], csb["gatew"][:])
                        gs = ep.tile([128, 1], F32, name="gs")
                        nc.vector.reduce_sum(gs[:], gm[:], axis=mybir.AxisListType.X)
                        eg = ep.tile([128, 1], F32, name="eg")
                        nc.scalar.activation(eg[:], gs[:], AFT.Exp, bias=csb["gateb"][:])
                        eg16 = ep.tile([128, 1], FP16, name="eg16")
                        nc.vector.tensor_copy(eg16[:], eg[:])
                        wg = ep.tile([128, G], FP16, name="wg")
                        nc.vector.tensor_mul(wg[:], bo_sb[:, gg, :],
                                             eg16[:].broadcast_to([128, G]))
                        stateC[gg] = (wg, eg16)

                def xl_stage(gg):
                    # PE consumers of the norm stage's outputs, one pipeline
                    # stage later so the PE never waits on the DVE norm chain.
                    nsl = slice(gg * 128, (gg + 1) * 128)
                    if li < 3:
                        hf = stateC.pop(gg)
                        psb = dxl.tile([128, 384], F32, name="psb")
                        for k in range(2):
                            nc.tensor.matmul(psb[:, 0:128], hf[:, k * 128:(k + 1) * 128],
                                             csb["id128"][:], start=True, stop=True)
                            nc.scalar.activation(hT_p[:, k, nsl], psb[:, 0:128], AFT.Copy)
                        for k in range(2):
                            nc.tensor.matmul(psb[:, 128:384], hT_p[:, k, nsl],
                                             wxl[li + 1][:, k, :],
                                             start=(k == 0), stop=(k == 1))
                        xls = ep.tile([128, 256], FP8, name="xls")
                        nc.scalar.activation(xls[:], psb[:, 128:384], AFT.Copy)
                        nc.sync.dma_start(xl_loc[nsl, :], xls[:])
                    else:
                        wg, eg16 = stateC.pop(gg)
                        psp = epp.tile([G, 257], F32, name="psp")
                        nc.tensor.matmul(psp[:, :256], wg[:], h_ln[:, gg, :],
                                         start=True, stop=True)
                        nc.tensor.matmul(psp[:, 256:257], bo_sb[:, gg, :], eg16[:],
                                         start=True, stop=True)
                        nc.vector.tensor_add(pre_acc[:], pre_acc[:], psp[:])

                for g0 in range(min(4, NGRP_USE)):
                    load_stage(g0)
                for gi in range(NGRP_USE + 3):
                    if gi + 4 < NGRP_USE:
                        load_stage(gi + 4)
                    if 1 <= gi <= NGRP_USE:
                        e23_stage(gi - 1)
                    if gi < NGRP_USE:
                        e1_stage(gi)
                    if 2 <= gi <= NGRP_USE + 1:
                        norm_stage(gi - 2)
                    if gi >= 3:
                        xl_stage(gi - 3)

        # ================= final: allreduce + transform =================
        with ExitStack() as lctx:
            fp_ = lctx.enter_context(tc.tile_pool(name="fin", bufs=1))
            fps = lctx.enter_context(tc.tile_pool(name="finps", bufs=2, space="PSUM"))
            nc.sync.dma_start(pre_in_d[:], pre_acc[:])
            nc.gpsimd.collective_compute(
                "AllReduce", ALU.add, replica_groups=RG,
                ins=[pre_in_d[:].opt()], outs=[pre_out_d[:].opt()])
            pre_all = fp_.tile([G, 257], F32)
            nc.sync.dma_start(pre_all[:], pre_out_d[:])
            preT = fp_.tile([128, 2, G], F32)
            for fb in range(2):
                pst = fps.tile([128, G], F32, name="pst")
                nc.tensor.matmul(pst[:], pre_all[:, fb * 128:(fb + 1) * 128],
                                 csb["id64"][:], start=True, stop=True)
                nc.vector.tensor_copy(preT[:, fb, :], pst[:])
            trw_sb = fp_.tile([128, 2, 256], F32)
            nc.sync.dma_start(trw_sb[:], cin["trw"][:].rearrange("f p m -> p f m"))
            pso = fps.tile([G, 256], F32, name="pso")
            for fb in range(2):
                nc.tensor.matmul(pso[:], preT[:, fb, :], trw_sb[:, fb, :],
                                 start=(fb == 0), stop=(fb == 1))
            deng = fp_.tile([G, 1], F32)
            nc.vector.tensor_copy(deng[:], pre_all[:, 256:257])
            recg = fp_.tile([G, 1], F32)
            nc.vector.reciprocal(recg[:], deng[:])
            outs = fp_.tile([G, 256], F32)
            nc.scalar.activation(outs[:], pso[:], AFT.Identity, scale=recg[:])
            outf = fp_.tile([G, 256], F32)
            nc.vector.tensor_add(outf[:], outs[:], csb["trb"][:])
            nc.sync.dma_start(out_t[:], outf[:])

    nc.compile()
    return nc


def build(inputs):
    host = _host_prep(inputs)
    egrp, nchk = host["egrp"], host["nchk"]
    key = (egrp, N_LAYERS, NGRP_USE, tuple(host["nchk_gs"]),
           _os.environ.get("K_ABL", ""))
    if key not in _prog_cache:
        _prog_cache[key] = _build_program(egrp, nchk,
                                          {li: host["wmeta"][li]["kb"] for li in (1, 2, 3)},
                                          host["nchk_gs"])
    nc = _prog_cache[key]

    in_maps = []
    for c in range(NCORE):
        hc = host["cores"][c]
        m = {
            "xT": hc["xT"], "src_idx": hc["src_idx"],
            "oh_em": hc["oh_em"], "oh_nm": hc["oh_nm"], "bonehot": hc["bonehot"],
        }
        for li in (1, 2, 3):
            wm = host["wmeta"][li]
            m[f"wcat{li}"] = wm["wcat"]
            m[f"webc{li}"] = np.ascontiguousarray(wm["webc"])
            m[f"attz{li}"] = np.ascontiguousarray(wm["attz"])
            m[f"nbias{li}"] = wm["nbias"]
        for k, v in host["consts"].items():
            m[k] = np.ascontiguousarray(v)
        in_maps.append(m)
    return nc, in_maps


def kernel(**inputs):
    nc, in_maps = build(inputs)
    res = run_bass_kernel_spmd(nc, in_maps, list(range(NCORE)))
    return np.asarray(res.results[0]["out"], np.float32)
